# revision 1
# baseline (speedup 1.0000x reference)
"""Trainium2 Bass kernel for nn_Convolution (gnn_message_passing).

Strategy (no collectives needed):
  - Edges are sorted by destination node and partitioned across the 8 cores
    by dst range (each core owns N/8 destination nodes). Node features and
    weights are replicated; each core redundantly computes the lin1 table
    for all nodes (phase 1), then processes only edges destined to its own
    node slice (phase 2) and writes its slice of the output.
  - Phase 2 runs in "windows" of 128 destination slots. Per window:
    gather y=l[src] rows with dma_gather, radial MLP on PE, message build on
    DVE (bf16), segment-sum via selection-matrix matmuls accumulating in
    PSUM, then one fused (lin2 + self-interaction) matmul chain.
  - All e3nn normalization constants and node_attr are folded into weights /
    edge attributes on the host.
"""

import sys

for _p in ("/opt/trn_rl_repo",):
    if _p not in sys.path:
        sys.path.insert(0, _p)

import numpy as np
import ml_dtypes

import concourse.bass as bass
import concourse.bacc as bacc
import concourse.mybir as mybir
import concourse.tile as tile
from concourse import bass_utils

BF16 = ml_dtypes.bfloat16

# Problem constants (hardcoded per contract)
N_NODES = 50000
N_EDGES = 800000
MUL0, MUL1 = 64, 32
N_BASIS, N_RADIAL = 10, 100
NUM_NEIGHBORS = 16.0
INV_SQRT3 = np.float32(1.0 / np.sqrt(3.0))
RELU_GAIN = np.float32(np.sqrt(2.0))
FAN_L2 = np.float32(np.sqrt(MUL0 + MUL1))

N_CORES = 8
SPLIT = 32768          # dma_gather idx is int16 -> split src tables
LROW = 256             # l-table row elems (bf16) -> 512B rows (256B-aligned)
WIN = 128              # dst slots per window

_LAST_RESULTS = None   # BassKernelResults of the most recent run (for test.py)


# --------------------------------------------------------------------------
# Device program
# --------------------------------------------------------------------------

def build_program(n_nodes, npc, n_win, t_a, t_b, num_cores, split=SPLIT):
    """Build the SPMD Bass program. npc = nodes per core."""
    tt = t_a + t_b
    ew = tt * 128            # padded edges per window
    e_core = n_win * ew
    f32, bf16, i16 = mybir.dt.float32, mybir.dt.bfloat16, mybir.dt.int16
    f32r = mybir.dt.float32r

    nc = bacc.Bacc("TRN2", target_bir_lowering=False, debug=False,
                   enable_asserts=False, num_devices=num_cores)

    # DRAM I/O (per-core data; weights replicated across cores)
    xaT = nc.dram_tensor("xaT", [160, n_nodes], f32, kind="ExternalInput").ap()
    xwin = nc.dram_tensor("xwin", [160, n_win * 128], f32, kind="ExternalInput").ap()
    embT = nc.dram_tensor("embT", [10, e_core], f32, kind="ExternalInput").ap()
    eattr = nc.dram_tensor("eattr", [128, n_win * tt * 4], bf16, kind="ExternalInput").ap()
    dstloc = nc.dram_tensor("dstloc", [128, n_win * tt], bf16, kind="ExternalInput").ap()
    idxA = nc.dram_tensor("idxA", [128, n_win * t_a * 8], i16, kind="ExternalInput").ap()
    idxB = nc.dram_tensor("idxB", [128, n_win * t_b * 8], i16, kind="ExternalInput").ap()
    Wl10 = nc.dram_tensor("Wl10", [64, 64], f32, kind="ExternalInput").ap()
    Wl11 = nc.dram_tensor("Wl11", [32, 32], f32, kind="ExternalInput").ap()
    Wfc1 = nc.dram_tensor("Wfc1", [10, 100], f32, kind="ExternalInput").ap()
    Wfc2b = nc.dram_tensor("Wfc2b", [100, 192], bf16, kind="ExternalInput").ap()
    Wbig = nc.dram_tensor("Wbig", [128, 5 * 256], f32, kind="ExternalInput").ap()
    iota = nc.dram_tensor("iota", [128, 128], bf16, kind="ExternalInput").ap()
    out = nc.dram_tensor("out", [n_win * 128, 160], f32, kind="ExternalOutput").ap()

    mult = mybir.AluOpType.mult
    addop = mybir.AluOpType.add
    iseq = mybir.AluOpType.is_equal
    relu = mybir.ActivationFunctionType.Relu

    with tile.TileContext(nc) as tc:
        with (
            tc.tile_pool(name="const", bufs=1) as cpool,
            tc.tile_pool(name="ld", bufs=2) as ldpool,
            tc.tile_pool(name="ltab", bufs=1, space="DRAM") as dpool,
            tc.tile_pool(name="win", bufs=2) as wpool,
            tc.tile_pool(name="scr", bufs=2) as spool,
        ):
            # ---- constants to SBUF
            wl10_sb = cpool.tile([64, 64], f32)
            nc.sync.dma_start(out=wl10_sb[:], in_=Wl10)
            wl11_sb = cpool.tile([32, 32], f32)
            nc.sync.dma_start(out=wl11_sb[:], in_=Wl11)
            wfc1_sb = cpool.tile([10, 100], f32)
            nc.sync.dma_start(out=wfc1_sb[:], in_=Wfc1)
            wfc2_sb = cpool.tile([100, 192], bf16)
            nc.sync.dma_start(out=wfc2_sb[:], in_=Wfc2b)
            wbig_sb = cpool.tile([128, 5 * 256], f32)
            nc.sync.dma_start(out=wbig_sb[:], in_=Wbig)
            iota_sb = cpool.tile([128, 128], bf16)
            nc.sync.dma_start(out=iota_sb[:], in_=iota)

            ltab = dpool.tile([n_nodes, LROW], bf16)

            # ---- phase 1: l table (lin1 of all nodes), bf16 rows in DRAM
            CH = 2048
            lps_ctx = tc.tile_pool(name="lps", bufs=2, space="PSUM")
            lpsum = lps_ctx.__enter__()
            for c0 in range(0, n_nodes, CH):
                cw = min(CH, n_nodes - c0)
                xa0 = ldpool.tile([64, CH], f32, tag="xa0")
                nc.sync.dma_start(out=xa0[:, :cw], in_=xaT[0:64, c0:c0 + cw])
                xa1 = [ldpool.tile([32, CH], f32, tag=f"xa1{d}", name=f"xa1{d}")
                       for d in range(3)]
                for d in range(3):
                    nc.sync.dma_start(out=xa1[d][:, :cw],
                                      in_=xaT[64 + 32 * d:96 + 32 * d, c0:c0 + cw])
                for t0 in range(0, cw, 128):
                    nn_ = min(128, cw - t0)
                    pl = lpsum.tile([128, 160], f32, tag="pl")
                    nc.tensor.matmul(out=pl[:nn_, 0:64],
                                     lhsT=xa0[:, t0:t0 + nn_],
                                     rhs=wl10_sb[:],
                                     start=True, stop=True)
                    for d in range(3):
                        nc.tensor.matmul(out=pl[:nn_, 64 + 32 * d:96 + 32 * d],
                                         lhsT=xa1[d][:, t0:t0 + nn_],
                                         rhs=wl11_sb[:],
                                         start=True, stop=True)
                    lt = ldpool.tile([128, 160], bf16, tag="lt")
                    nc.vector.tensor_copy(out=lt[:nn_, :], in_=pl[:nn_, :])
                    nc.sync.dma_start(out=ltab[c0 + t0:c0 + t0 + nn_, 0:160],
                                      in_=lt[:nn_, :])

            lps_ctx.__exit__(None, None, None)

            # ---- phase 2: windows
            ps_ctx = tc.tile_pool(name="ps", bufs=2, space="PSUM")
            psum = ps_ctx.__enter__()
            n5 = (tt * 128 + 511) // 512
            for w in range(n_win):
                # loads
                ixa_w = wpool.tile([128, t_a * 8], i16, tag="ixa")
                nc.sync.dma_start(out=ixa_w[:], in_=idxA[:, w * t_a * 8:(w + 1) * t_a * 8])
                ixb_w = wpool.tile([128, t_b * 8], i16, tag="ixb")
                nc.sync.dma_start(out=ixb_w[:], in_=idxB[:, w * t_b * 8:(w + 1) * t_b * 8])
                y_w = wpool.tile([128, tt * 256], bf16, tag="y")
                nc.gpsimd.dma_gather(
                    y_w[:].rearrange("p (t e) -> p t e", e=256)[:, 0:t_a, :],
                    ltab[:],
                    ixa_w[:],
                    t_a * 128, t_a * 128, 256, single_packet=False)
                nc.gpsimd.dma_gather(
                    y_w[:].rearrange("p (t e) -> p t e", e=256)[:, t_a:tt, :],
                    ltab[:][split:n_nodes, :],
                    ixb_w[:],
                    t_b * 128, t_b * 128, 256, single_packet=False)
                emb_w = wpool.tile([10, tt * 128], f32, tag="emb")
                nc.sync.dma_start(out=emb_w[:], in_=embT[:, w * ew:(w + 1) * ew])
                ea_w = wpool.tile([128, tt * 4], bf16, tag="ea")
                nc.sync.dma_start(out=ea_w[:], in_=eattr[:, w * tt * 4:(w + 1) * tt * 4])
                dl_w = wpool.tile([128, tt], bf16, tag="dl")
                nc.sync.dma_start(out=dl_w[:], in_=dstloc[:, w * tt:(w + 1) * tt])
                xw_a = wpool.tile([128, 128], f32, tag="xwa")
                nc.sync.dma_start(out=xw_a[:], in_=xwin[0:128, w * 128:(w + 1) * 128])
                xw_b = wpool.tile([32, 128], f32, tag="xwb")
                nc.sync.dma_start(out=xw_b[:], in_=xwin[128:160, w * 128:(w + 1) * 128])

                # radial MLP layer 1 (fp32r), relu -> bf16
                hT = wpool.tile([100, tt * 128], bf16, tag="hT")
                for c5 in range(n5):
                    ne = min(512, tt * 128 - c5 * 512)
                    ph = psum.tile([100, 512], f32, tag="ph")
                    nc.tensor.matmul(out=ph[:, :ne],
                                     lhsT=wfc1_sb[:],
                                     rhs=emb_w[:, c5 * 512:c5 * 512 + ne],
                                     start=True, stop=True)
                    nc.scalar.activation(hT[:, c5 * 512:c5 * 512 + ne], ph[:, :ne], relu)

                # radial layer 2 (bf16) per edge tile
                w_w = wpool.tile([128, tt * 192], bf16, tag="ww")
                for t in range(tt):
                    pw = psum.tile([128, 192], f32, tag="pw")
                    nc.tensor.matmul(out=pw[:],
                                     lhsT=hT[:, t * 128:(t + 1) * 128],
                                     rhs=wfc2_sb[:], start=True, stop=True)
                    nc.vector.tensor_copy(out=w_w[:, t * 192:(t + 1) * 192], in_=pw[:])

                # selection matrices A (bf16 0/1) per tile
                A_w = wpool.tile([128, tt * 128], bf16, tag="A")
                for t in range(tt):
                    nc.vector.tensor_tensor(
                        out=A_w[:, t * 128:(t + 1) * 128],
                        in0=dl_w[:, t:t + 1].to_broadcast([128, 128]),
                        in1=iota_sb[:], op=iseq)

                # messages M [128, tt, 384] bf16
                M_w = wpool.tile([128, tt * 384], bf16, tag="M")
                y3 = y_w[:].rearrange("p (t e) -> p t e", e=256)
                w3 = w_w[:].rearrange("p (t e) -> p t e", e=192)
                m3 = M_w[:].rearrange("p (t e) -> p t e", e=384)
                ea3 = ea_w[:].rearrange("p (t e) -> p t e", e=4)

                def eb(col, n):
                    return ea3[:, :, col:col + 1].to_broadcast([128, tt, n])

                t0_s = spool.tile([128, tt * 64], bf16, tag="t0")
                t0v = t0_s[:].rearrange("p (t e) -> p t e", e=64)
                t1_s = spool.tile([128, tt * 64], bf16, tag="t1")
                t1v = t1_s[:].rearrange("p (t e) -> p t e", e=64)
                t2_s = spool.tile([128, tt * 32], bf16, tag="t2")
                t2v = t2_s[:].rearrange("p (t e) -> p t e", e=32)
                z_s = spool.tile([128, tt * 96], bf16, tag="z")
                zv = z_s[:].rearrange("p (t e) -> p t e", e=96)
                zz_s = spool.tile([128, tt * 32], bf16, tag="zz")
                zzv = zz_s[:].rearrange("p (t e) -> p t e", e=32)

                tt_ = nc.vector.tensor_tensor
                # m0 = (w0*y0)*e0
                tt_(out=t0v, in0=w3[:, :, 0:64], in1=y3[:, :, 0:64], op=mult)
                tt_(out=m3[:, :, 0:64], in0=t0v, in1=eb(0, 64), op=mult)
                # m1_d = (w1*y0)*e1d
                tt_(out=t1v, in0=w3[:, :, 64:128], in1=y3[:, :, 0:64], op=mult)
                for d in range(3):
                    tt_(out=m3[:, :, 64 + 64 * d:128 + 64 * d],
                        in0=t1v, in1=eb(1 + d, 64), op=mult)
                # m2_d = (w2*e0)*y1_d
                tt_(out=t2v, in0=w3[:, :, 128:160], in1=eb(0, 32), op=mult)
                for d in range(3):
                    tt_(out=m3[:, :, 256 + 32 * d:288 + 32 * d],
                        in0=t2v, in1=y3[:, :, 64 + 32 * d:96 + 32 * d], op=mult)
                # m3 = w3 * sum_d(y1_d*e1_d)
                for d in range(3):
                    tt_(out=zv[:, :, 32 * d:32 * (d + 1)],
                        in0=y3[:, :, 64 + 32 * d:96 + 32 * d], in1=eb(1 + d, 32), op=mult)
                tt_(out=zzv, in0=zv[:, :, 0:32], in1=zv[:, :, 32:64], op=addop)
                tt_(out=zzv, in0=zzv, in1=zv[:, :, 64:96], op=addop)
                tt_(out=m3[:, :, 352:384], in0=zzv, in1=w3[:, :, 160:192], op=mult)

                # segment-sum: sT[f, slot] += M_chunk.T @ A  (3 chunks, acc over t)
                pst = psum.tile([128, 384], f32, tag="pst")
                for ch in range(3):
                    for t in range(tt):
                        nc.tensor.matmul(
                            out=pst[:, ch * 128:(ch + 1) * 128],
                            lhsT=m3[:, t, ch * 128:(ch + 1) * 128],
                            rhs=A_w[:, t * 128:(t + 1) * 128],
                            start=(t == 0), stop=(t == tt - 1))
                sT_sb = spool.tile([128, 384], f32, tag="sT")
                nc.vector.tensor_copy(out=sT_sb[:], in_=pst[:])

                # fused lin2 + self-interaction: out[slot, 0:160]
                po = psum.tile([128, 256], f32, tag="po")
                for ch in range(3):
                    nc.tensor.matmul(out=po[:],
                                     lhsT=sT_sb[:, ch * 128:(ch + 1) * 128],
                                     rhs=wbig_sb[:, ch * 256:(ch + 1) * 256],
                                     start=(ch == 0), stop=False)
                nc.tensor.matmul(out=po[:], lhsT=xw_a[:],
                                 rhs=wbig_sb[:, 768:1024],
                                 start=False, stop=False)
                nc.tensor.matmul(out=po[:], lhsT=xw_b[:],
                                 rhs=wbig_sb[0:32, 1024:1280],
                                 start=False, stop=True)
                o_sb = spool.tile([128, 160], f32, tag="o")
                nc.vector.tensor_copy(out=o_sb[:], in_=po[:, 0:160])
                nc.sync.dma_start(out=out[w * 128:(w + 1) * 128, :], in_=o_sb[:])
            ps_ctx.__exit__(None, None, None)

    nc.compile()
    return nc


# --------------------------------------------------------------------------
# Host-side preparation
# --------------------------------------------------------------------------

def prepare(inputs, n_nodes=N_NODES, num_cores=N_CORES, split=SPLIT):
    npc = n_nodes // num_cores
    n_win = (npc + WIN - 1) // WIN

    f32 = np.float32
    node_input = np.asarray(inputs["node_input"], f32)
    node_attr = np.asarray(inputs["node_attr"], f32)
    edge_attr = np.asarray(inputs["edge_attr"], f32)
    emb = np.asarray(inputs["edge_length_embedded"], f32)
    src = np.asarray(inputs["edge_src"], np.int64)
    dst = np.asarray(inputs["edge_dst"], np.int64)
    E = src.shape[0]

    # fold node_attr into node features; de-interleave x1 by d
    xa = node_input * node_attr
    xg = np.concatenate([xa[:, :MUL0], xa[:, MUL0 + 0::3],
                         xa[:, MUL0 + 1::3], xa[:, MUL0 + 2::3]], axis=1)
    xaT = np.ascontiguousarray(xg.T)                      # [160, n_nodes]

    # fold node_attr[dst] into edge_attr
    eattr_f = edge_attr * node_attr[dst, 0][:, None]

    # weights with norm constants folded
    Wl10 = np.asarray(inputs["W_l1_0"], f32) / np.sqrt(MUL0).astype(f32)
    Wl11 = np.asarray(inputs["W_l1_1"], f32) / np.sqrt(MUL1).astype(f32)
    Wfc1 = np.asarray(inputs["W_fc1"], f32) / np.sqrt(np.float32(N_BASIS))
    Wfc2b = (np.asarray(inputs["W_fc2"], f32) * (RELU_GAIN / np.sqrt(np.float32(N_RADIAL)))).astype(BF16)

    c2 = np.float32(0.5 / np.sqrt(NUM_NEIGHBORS) / FAN_L2)
    W2cat = np.zeros((384, 256), f32)
    W2cat[0:64, 0:64] = np.asarray(inputs["W_l2_00"], f32) * c2
    W2cat[352:384, 0:64] = np.asarray(inputs["W_l2_10"], f32) * c2 * INV_SQRT3
    for d in range(3):
        W2cat[64 + 64 * d:128 + 64 * d, 64 + 32 * d:96 + 32 * d] = \
            np.asarray(inputs["W_l2_01"], f32) * c2
        W2cat[256 + 32 * d:288 + 32 * d, 64 + 32 * d:96 + 32 * d] = \
            np.asarray(inputs["W_l2_11"], f32) * c2
    Wsi = np.zeros((160, 256), f32)
    Wsi[0:64, 0:64] = np.asarray(inputs["W_si0"], f32) / np.sqrt(MUL0).astype(f32)
    for d in range(3):
        Wsi[64 + 32 * d:96 + 32 * d, 64 + 32 * d:96 + 32 * d] = \
            np.asarray(inputs["W_si1"], f32) / np.sqrt(MUL1).astype(f32)
    Wfull = np.vstack([W2cat, Wsi])                       # [544, 256]
    Wbig = np.zeros((128, 5 * 256), f32)
    for ch in range(4):
        Wbig[:, ch * 256:(ch + 1) * 256] = Wfull[ch * 128:(ch + 1) * 128]
    Wbig[0:32, 1024:1280] = Wfull[512:544]

    iota = np.broadcast_to(np.arange(128, dtype=f32), (128, 128)).astype(BF16)

    # ---- edge partition: (core, window, srcblock), stable sorted
    core = dst // npc
    dloc = dst - core * npc
    win = dloc // WIN
    slot = dloc % WIN
    isA = (src < split).astype(np.int64)
    nk = num_cores * n_win * 2
    key = (core * n_win + win) * 2 + (1 - isA)
    order = np.argsort(key, kind="stable")
    sk = key[order]
    cnt = np.bincount(key, minlength=nk)
    cntA = cnt[0::2].reshape(num_cores, n_win)
    cntB = cnt[1::2].reshape(num_cores, n_win)
    t_a = max(1, int(-(-cntA.max() // 128)))
    t_b = max(1, int(-(-cntB.max() // 128)))
    tt = t_a + t_b
    ew = tt * 128
    e_core = n_win * ew

    grp_start = np.searchsorted(sk, np.arange(nk))
    pos = np.arange(E) - grp_start[sk]
    c_s = sk // (n_win * 2)
    w_s = (sk // 2) % n_win
    b_s = sk % 2
    dstpos = c_s * e_core + w_s * ew + b_s * (t_a * 128) + pos

    perm = np.full(num_cores * e_core, -1, np.int64)
    perm[dstpos] = order
    valid = perm >= 0
    pidx = np.where(valid, perm, 0)

    emb_p = (emb[pidx] * valid[:, None]).astype(f32)      # [8EC, 10]
    ea_p = (eattr_f[pidx] * valid[:, None]).astype(BF16)  # [8EC, 4]
    blockpat = np.concatenate([np.zeros(t_a * 128, np.int64),
                               np.ones(t_b * 128, np.int64)])
    blockpat = np.tile(blockpat, num_cores * n_win)
    iv = np.where(valid, src[pidx] - split * blockpat, 0).astype(np.int16)
    sl_p = np.where(valid, slot[pidx], 0).astype(BF16)

    def percore(a):
        return a

    # device layouts
    embT_c = emb_p.reshape(num_cores, e_core, 10).transpose(0, 2, 1).copy()
    ea_c = ea_p.reshape(num_cores, n_win, tt, 128, 4).transpose(0, 3, 1, 2, 4) \
        .reshape(num_cores, 128, n_win * tt * 4).copy()
    dl_c = sl_p.reshape(num_cores, n_win, tt, 128).transpose(0, 3, 1, 2) \
        .reshape(num_cores, 128, n_win * tt).copy()
    ivr = iv.reshape(num_cores, n_win, tt * 128)
    ivA = ivr[:, :, :t_a * 128].reshape(num_cores, n_win, t_a * 8, 16)
    idxA_c = np.tile(ivA.transpose(0, 3, 1, 2).reshape(num_cores, 16, n_win * t_a * 8),
                     (1, 8, 1)).copy()
    ivB = ivr[:, :, t_a * 128:].reshape(num_cores, n_win, t_b * 8, 16)
    idxB_c = np.tile(ivB.transpose(0, 3, 1, 2).reshape(num_cores, 16, n_win * t_b * 8),
                     (1, 8, 1)).copy()

    xwin_c = np.zeros((num_cores, 160, n_win * 128), f32)
    for c in range(num_cores):
        xwin_c[c, :, :npc] = xaT[:, c * npc:(c + 1) * npc]

    in_maps = []
    for c in range(num_cores):
        in_maps.append({
            "xaT": xaT, "xwin": xwin_c[c],
            "embT": embT_c[c], "eattr": ea_c[c], "dstloc": dl_c[c],
            "idxA": idxA_c[c], "idxB": idxB_c[c],
            "Wl10": Wl10, "Wl11": Wl11, "Wfc1": Wfc1,
            "Wfc2b": Wfc2b, "Wbig": Wbig, "iota": iota,
        })
    meta = dict(n_nodes=n_nodes, npc=npc, n_win=n_win, t_a=t_a, t_b=t_b,
                num_cores=num_cores, split=split)
    return in_maps, meta


def assemble(results, meta):
    """results: list of per-core dicts with 'out' [n_win*128,160] deint."""
    npc = meta["npc"]
    full = np.concatenate([r["out"][:npc] for r in results], axis=0)
    out = np.empty_like(full)
    out[:, :MUL0] = full[:, :MUL0]
    for d in range(3):
        out[:, MUL0 + d::3] = full[:, MUL0 + 32 * d:MUL0 + 32 * (d + 1)]
    return np.ascontiguousarray(out, dtype=np.float32)


_LAST_NC = None
_LAST_INMAPS = None
_LAST_META = None


def kernel(**inputs):
    global _LAST_RESULTS, _LAST_NC, _LAST_INMAPS, _LAST_META
    in_maps, meta = prepare(inputs)
    nc = build_program(meta["n_nodes"], meta["npc"], meta["n_win"],
                       meta["t_a"], meta["t_b"], meta["num_cores"],
                       split=meta["split"])
    _LAST_NC, _LAST_INMAPS, _LAST_META = nc, in_maps, meta
    res = bass_utils.run_bass_kernel_spmd(
        nc, in_maps, core_ids=list(range(meta["num_cores"])))
    _LAST_RESULTS = res
    return assemble(res.results, meta)



# revision 2
# speedup vs baseline: 1.2078x; 1.2078x over previous
"""Trainium2 Bass kernel for nn_Convolution (gnn_message_passing).

Strategy (no collectives needed):
  - Edges are sorted by destination node and partitioned across the 8 cores
    by dst range (each core owns N/8 destination nodes). Node features and
    weights are replicated; each core redundantly computes the lin1 table
    for all nodes (phase 1), then processes only edges destined to its own
    node slice (phase 2) and writes its slice of the output.
  - Phase 2 runs in "windows" of 128 destination slots. Per window:
    gather y=l[src] rows with dma_gather, radial MLP on PE, message build on
    DVE (bf16), segment-sum via selection-matrix matmuls accumulating in
    PSUM, then one fused (lin2 + self-interaction) matmul chain.
  - All matmuls are bf16 (fp32 LOW_HIGH matmuls are ~10x slower on PE).
  - All e3nn normalization constants and node_attr are folded into weights /
    edge attributes on the host.
"""

import sys

for _p in ("/opt/trn_rl_repo",):
    if _p not in sys.path:
        sys.path.insert(0, _p)

import numpy as np
import ml_dtypes

import concourse.bass as bass
import concourse.bacc as bacc
import concourse.mybir as mybir
import concourse.tile as tile
from concourse import bass_utils

BF16 = ml_dtypes.bfloat16

# Problem constants (hardcoded per contract)
N_NODES = 50000
N_EDGES = 800000
MUL0, MUL1 = 64, 32
N_BASIS, N_RADIAL = 10, 100
NUM_NEIGHBORS = 16.0
INV_SQRT3 = np.float32(1.0 / np.sqrt(3.0))
RELU_GAIN = np.float32(np.sqrt(2.0))
FAN_L2 = np.float32(np.sqrt(MUL0 + MUL1))

N_CORES = 8
SPLIT = 32768          # dma_gather idx is int16 -> split src tables
LROW = 256             # l-table row elems (bf16) -> 512B rows (256B-aligned)
WIN = 128              # dst slots per window

_LAST_RESULTS = None   # BassKernelResults of the most recent run (for test.py)


# --------------------------------------------------------------------------
# Device program
# --------------------------------------------------------------------------

def build_program(n_nodes, npc, n_win, t_a, t_b, num_cores, split=SPLIT):
    """Build the SPMD Bass program. npc = nodes per core."""
    tt = t_a + t_b
    ew = tt * 128            # padded edges per window
    e_core = n_win * ew
    mcols = tt * 5 + tt * 8  # meta: ea(tt*4) + dl(tt) + idxA(t_a*8) + idxB(t_b*8)
    oA = tt * 5
    oB = tt * 5 + t_a * 8
    f32, bf16, i16 = mybir.dt.float32, mybir.dt.bfloat16, mybir.dt.int16

    nc = bacc.Bacc("TRN2", target_bir_lowering=False, debug=False,
                   enable_asserts=False, num_devices=num_cores)

    # DRAM I/O (per-core data; weights replicated across cores)
    xaT = nc.dram_tensor("xaT", [160, n_nodes], bf16, kind="ExternalInput").ap()
    xwin = nc.dram_tensor("xwin", [160, n_win * 128], bf16, kind="ExternalInput").ap()
    embT = nc.dram_tensor("embT", [10, e_core], bf16, kind="ExternalInput").ap()
    meta = nc.dram_tensor("meta", [128, n_win * mcols], i16, kind="ExternalInput").ap()
    Wbd = nc.dram_tensor("Wbd", [128, 160], bf16, kind="ExternalInput").ap()
    W2b = nc.dram_tensor("W2b", [32, 160], bf16, kind="ExternalInput").ap()
    Wfc1 = nc.dram_tensor("Wfc1", [10, 100], bf16, kind="ExternalInput").ap()
    Wfc2b = nc.dram_tensor("Wfc2b", [100, 192], bf16, kind="ExternalInput").ap()
    Wbig = nc.dram_tensor("Wbig", [128, 5 * 160], bf16, kind="ExternalInput").ap()
    iota = nc.dram_tensor("iota", [128, 128], bf16, kind="ExternalInput").ap()
    out = nc.dram_tensor("out", [n_win * 128, 160], f32, kind="ExternalOutput").ap()

    mult = mybir.AluOpType.mult
    addop = mybir.AluOpType.add
    iseq = mybir.AluOpType.is_equal
    relu = mybir.ActivationFunctionType.Relu

    with tile.TileContext(nc) as tc:
        with (
            tc.tile_pool(name="const", bufs=1) as cpool,
            tc.tile_pool(name="ld", bufs=2) as ldpool,
            tc.tile_pool(name="ltab", bufs=1, space="DRAM") as dpool,
            tc.tile_pool(name="win", bufs=3) as wpool,
            tc.tile_pool(name="scr", bufs=2) as spool,
        ):
            # ---- constants to SBUF
            wbd_sb = cpool.tile([128, 160], bf16)
            nc.sync.dma_start(out=wbd_sb[:], in_=Wbd)
            w2b_sb = cpool.tile([32, 160], bf16)
            nc.sync.dma_start(out=w2b_sb[:], in_=W2b)
            wfc1_sb = cpool.tile([10, 100], bf16)
            nc.sync.dma_start(out=wfc1_sb[:], in_=Wfc1)
            wfc2_sb = cpool.tile([100, 192], bf16)
            nc.sync.dma_start(out=wfc2_sb[:], in_=Wfc2b)
            wbig_sb = cpool.tile([128, 5 * 160], bf16)
            nc.sync.dma_start(out=wbig_sb[:], in_=Wbig)
            iota_sb = cpool.tile([128, 128], bf16)
            nc.sync.dma_start(out=iota_sb[:], in_=iota)

            ltab = dpool.tile([n_nodes, LROW], bf16)

            # ---- phase 1: l table (lin1 of all nodes), bf16 rows in DRAM
            # per 128-node tile: 2 fused matmuls with block-diagonal weights
            CH = 4096
            lps_ctx = tc.tile_pool(name="lps", bufs=2, space="PSUM")
            lpsum = lps_ctx.__enter__()
            for c0 in range(0, n_nodes, CH):
                cw = min(CH, n_nodes - c0)
                xa = ldpool.tile([128, CH], bf16, tag="xa")
                nc.sync.dma_start(out=xa[:, :cw], in_=xaT[0:128, c0:c0 + cw])
                xb = ldpool.tile([32, CH], bf16, tag="xb")
                nc.sync.dma_start(out=xb[:, :cw], in_=xaT[128:160, c0:c0 + cw])
                for t0 in range(0, cw, 128):
                    nn_ = min(128, cw - t0)
                    pl = lpsum.tile([128, 160], f32, tag="pl")
                    nc.tensor.matmul(out=pl[:nn_, :],
                                     lhsT=xa[:, t0:t0 + nn_],
                                     rhs=wbd_sb[:],
                                     start=True, stop=False)
                    nc.tensor.matmul(out=pl[:nn_, :],
                                     lhsT=xb[:, t0:t0 + nn_],
                                     rhs=w2b_sb[:],
                                     start=False, stop=True)
                    lt = ldpool.tile([128, 160], bf16, tag="lt")
                    nc.scalar.copy(out=lt[:nn_, :], in_=pl[:nn_, :])
                    nc.sync.dma_start(out=ltab[c0 + t0:c0 + t0 + nn_, 0:160],
                                      in_=lt[:nn_, :])
            lps_ctx.__exit__(None, None, None)

            # ---- phase 2: windows
            ps_ctx = tc.tile_pool(name="ps", bufs=2, space="PSUM")
            psum = ps_ctx.__enter__()
            n5 = (tt * 128 + 511) // 512
            for w in range(n_win):
                # loads (meta packs ea/dl/idxA/idxB in one i16 tensor)
                meta_w = wpool.tile([128, mcols], i16, tag="meta")
                nc.sync.dma_start(out=meta_w[:], in_=meta[:, w * mcols:(w + 1) * mcols])
                ea_w = meta_w[:, 0:tt * 4].bitcast(bf16)
                dl_w = meta_w[:, tt * 4:tt * 5].bitcast(bf16)
                y_w = wpool.tile([128, tt * 256], bf16, tag="y")
                nc.gpsimd.dma_gather(
                    y_w[:].rearrange("p (t e) -> p t e", e=256)[:, 0:t_a, :],
                    ltab[:],
                    meta_w[:, oA:oA + t_a * 8],
                    t_a * 128, t_a * 128, 256, single_packet=False)
                nc.gpsimd.dma_gather(
                    y_w[:].rearrange("p (t e) -> p t e", e=256)[:, t_a:tt, :],
                    ltab[:][split:n_nodes, :],
                    meta_w[:, oB:oB + t_b * 8],
                    t_b * 128, t_b * 128, 256, single_packet=False)
                emb_w = wpool.tile([10, tt * 128], bf16, tag="emb")
                nc.sync.dma_start(out=emb_w[:], in_=embT[:, w * ew:(w + 1) * ew])
                xw_a = wpool.tile([128, 128], bf16, tag="xwa")
                nc.sync.dma_start(out=xw_a[:], in_=xwin[0:128, w * 128:(w + 1) * 128])
                xw_b = wpool.tile([32, 128], bf16, tag="xwb")
                nc.sync.dma_start(out=xw_b[:], in_=xwin[128:160, w * 128:(w + 1) * 128])

                # radial MLP layer 1 (bf16), relu -> bf16
                hT = spool.tile([100, tt * 128], bf16, tag="hT")
                for c5 in range(n5):
                    ne = min(512, tt * 128 - c5 * 512)
                    ph = psum.tile([100, 512], f32, tag="ph")
                    nc.tensor.matmul(out=ph[:, :ne],
                                     lhsT=wfc1_sb[:],
                                     rhs=emb_w[:, c5 * 512:c5 * 512 + ne],
                                     start=True, stop=True)
                    nc.scalar.activation(hT[:, c5 * 512:c5 * 512 + ne], ph[:, :ne], relu)

                # radial layer 2 (bf16) per edge tile
                w_w = spool.tile([128, tt * 192], bf16, tag="ww")
                for t in range(tt):
                    pw = psum.tile([128, 192], f32, tag="pw")
                    nc.tensor.matmul(out=pw[:],
                                     lhsT=hT[:, t * 128:(t + 1) * 128],
                                     rhs=wfc2_sb[:], start=True, stop=True)
                    nc.vector.tensor_copy(out=w_w[:, t * 192:(t + 1) * 192], in_=pw[:])

                # selection matrices A (bf16 0/1), one batched iseq
                A_w = spool.tile([128, tt * 128], bf16, tag="A")
                nc.vector.tensor_tensor(
                    out=A_w[:].rearrange("p (t n) -> p t n", n=128),
                    in0=dl_w.rearrange("p (t o) -> p t o", o=1)
                        .to_broadcast([128, tt, 128]),
                    in1=iota_sb[:].rearrange("p (o n) -> p o n", o=1)
                        .to_broadcast([128, tt, 128]),
                    op=iseq)

                # messages M [128, tt, 384] bf16
                M_w = spool.tile([128, tt * 384], bf16, tag="M")
                y3 = y_w[:].rearrange("p (t e) -> p t e", e=256)
                w3 = w_w[:].rearrange("p (t e) -> p t e", e=192)
                m3 = M_w[:].rearrange("p (t e) -> p t e", e=384)
                ea3 = ea_w.rearrange("p (t e) -> p t e", e=4)

                def eb(col, n):
                    return ea3[:, :, col:col + 1].to_broadcast([128, tt, n])

                t0_s = spool.tile([128, tt * 64], bf16, tag="t0")
                t0v = t0_s[:].rearrange("p (t e) -> p t e", e=64)
                t1_s = spool.tile([128, tt * 64], bf16, tag="t1")
                t1v = t1_s[:].rearrange("p (t e) -> p t e", e=64)
                t2_s = spool.tile([128, tt * 32], bf16, tag="t2")
                t2v = t2_s[:].rearrange("p (t e) -> p t e", e=32)
                z_s = spool.tile([128, tt * 96], bf16, tag="z")
                zv = z_s[:].rearrange("p (t e) -> p t e", e=96)
                zz_s = spool.tile([128, tt * 32], bf16, tag="zz")
                zzv = zz_s[:].rearrange("p (t e) -> p t e", e=32)

                tt_ = nc.vector.tensor_tensor
                # m0 = (w0*y0)*e0
                tt_(out=t0v, in0=w3[:, :, 0:64], in1=y3[:, :, 0:64], op=mult)
                tt_(out=m3[:, :, 0:64], in0=t0v, in1=eb(0, 64), op=mult)
                # m1_d = (w1*y0)*e1d
                tt_(out=t1v, in0=w3[:, :, 64:128], in1=y3[:, :, 0:64], op=mult)
                for d in range(3):
                    tt_(out=m3[:, :, 64 + 64 * d:128 + 64 * d],
                        in0=t1v, in1=eb(1 + d, 64), op=mult)
                # m2_d = (w2*e0)*y1_d
                tt_(out=t2v, in0=w3[:, :, 128:160], in1=eb(0, 32), op=mult)
                for d in range(3):
                    tt_(out=m3[:, :, 256 + 32 * d:288 + 32 * d],
                        in0=t2v, in1=y3[:, :, 64 + 32 * d:96 + 32 * d], op=mult)
                # m3 = w3 * sum_d(y1_d*e1_d)
                for d in range(3):
                    tt_(out=zv[:, :, 32 * d:32 * (d + 1)],
                        in0=y3[:, :, 64 + 32 * d:96 + 32 * d], in1=eb(1 + d, 32), op=mult)
                tt_(out=zzv, in0=zv[:, :, 0:32], in1=zv[:, :, 32:64], op=addop)
                tt_(out=zzv, in0=zzv, in1=zv[:, :, 64:96], op=addop)
                tt_(out=m3[:, :, 352:384], in0=zzv, in1=w3[:, :, 160:192], op=mult)

                # segment-sum: sT[f, slot] += M_chunk.T @ A  (3 chunks, acc over t)
                pst = psum.tile([128, 384], f32, tag="pst")
                for ch in range(3):
                    for t in range(tt):
                        nc.tensor.matmul(
                            out=pst[:, ch * 128:(ch + 1) * 128],
                            lhsT=m3[:, t, ch * 128:(ch + 1) * 128],
                            rhs=A_w[:, t * 128:(t + 1) * 128],
                            start=(t == 0), stop=(t == tt - 1))
                sT_sb = spool.tile([128, 384], bf16, tag="sT")
                nc.vector.tensor_copy(out=sT_sb[:], in_=pst[:])

                # fused lin2 + self-interaction: out[slot, 0:160], all bf16
                po = psum.tile([128, 160], f32, tag="po")
                for ch in range(3):
                    nc.tensor.matmul(out=po[:],
                                     lhsT=sT_sb[:, ch * 128:(ch + 1) * 128],
                                     rhs=wbig_sb[:, ch * 160:(ch + 1) * 160],
                                     start=(ch == 0), stop=False)
                nc.tensor.matmul(out=po[:], lhsT=xw_a[:],
                                 rhs=wbig_sb[:, 480:640],
                                 start=False, stop=False)
                nc.tensor.matmul(out=po[:], lhsT=xw_b[:],
                                 rhs=wbig_sb[0:32, 640:800],
                                 start=False, stop=True)
                o_sb = spool.tile([128, 160], f32, tag="o")
                nc.vector.tensor_copy(out=o_sb[:], in_=po[:])
                nc.sync.dma_start(out=out[w * 128:(w + 1) * 128, :], in_=o_sb[:])
            ps_ctx.__exit__(None, None, None)

    nc.compile()
    return nc


# --------------------------------------------------------------------------
# Host-side preparation
# --------------------------------------------------------------------------

def prepare(inputs, n_nodes=N_NODES, num_cores=N_CORES, split=SPLIT):
    npc = n_nodes // num_cores
    n_win = (npc + WIN - 1) // WIN

    f32 = np.float32
    node_input = np.asarray(inputs["node_input"], f32)
    node_attr = np.asarray(inputs["node_attr"], f32)
    edge_attr = np.asarray(inputs["edge_attr"], f32)
    emb = np.asarray(inputs["edge_length_embedded"], f32)
    src = np.asarray(inputs["edge_src"], np.int64)
    dst = np.asarray(inputs["edge_dst"], np.int64)
    E = src.shape[0]

    # fold node_attr into node features; de-interleave x1 by d
    xa = node_input * node_attr
    xg = np.concatenate([xa[:, :MUL0], xa[:, MUL0 + 0::3],
                         xa[:, MUL0 + 1::3], xa[:, MUL0 + 2::3]], axis=1)
    xaT = np.ascontiguousarray(xg.T).astype(BF16)         # [160, n_nodes]

    # fold node_attr[dst] into edge_attr
    eattr_f = edge_attr * node_attr[dst, 0][:, None]

    # weights with norm constants folded; phase-1 block-diagonal layout
    Wl10 = np.asarray(inputs["W_l1_0"], f32) / np.sqrt(MUL0).astype(f32)
    Wl11 = np.asarray(inputs["W_l1_1"], f32) / np.sqrt(MUL1).astype(f32)
    Wbd = np.zeros((128, 160), f32)
    Wbd[0:64, 0:64] = Wl10
    Wbd[64:96, 64:96] = Wl11
    Wbd[96:128, 96:128] = Wl11
    W2b = np.zeros((32, 160), f32)
    W2b[:, 128:160] = Wl11
    Wfc1 = (np.asarray(inputs["W_fc1"], f32) / np.sqrt(np.float32(N_BASIS))).astype(BF16)
    Wfc2b = (np.asarray(inputs["W_fc2"], f32) * (RELU_GAIN / np.sqrt(np.float32(N_RADIAL)))).astype(BF16)

    c2 = np.float32(0.5 / np.sqrt(NUM_NEIGHBORS) / FAN_L2)
    W2cat = np.zeros((384, 160), f32)
    W2cat[0:64, 0:64] = np.asarray(inputs["W_l2_00"], f32) * c2
    W2cat[352:384, 0:64] = np.asarray(inputs["W_l2_10"], f32) * c2 * INV_SQRT3
    for d in range(3):
        W2cat[64 + 64 * d:128 + 64 * d, 64 + 32 * d:96 + 32 * d] = \
            np.asarray(inputs["W_l2_01"], f32) * c2
        W2cat[256 + 32 * d:288 + 32 * d, 64 + 32 * d:96 + 32 * d] = \
            np.asarray(inputs["W_l2_11"], f32) * c2
    Wsi = np.zeros((160, 160), f32)
    Wsi[0:64, 0:64] = np.asarray(inputs["W_si0"], f32) / np.sqrt(MUL0).astype(f32)
    for d in range(3):
        Wsi[64 + 32 * d:96 + 32 * d, 64 + 32 * d:96 + 32 * d] = \
            np.asarray(inputs["W_si1"], f32) / np.sqrt(MUL1).astype(f32)
    Wfull = np.vstack([W2cat, Wsi])                       # [544, 160]
    Wbig = np.zeros((128, 5 * 160), f32)
    for ch in range(4):
        Wbig[:, ch * 160:(ch + 1) * 160] = Wfull[ch * 128:(ch + 1) * 128]
    Wbig[0:32, 640:800] = Wfull[512:544]

    iota = np.broadcast_to(np.arange(128, dtype=f32), (128, 128)).astype(BF16)

    # ---- edge partition: (core, window, srcblock), stable sorted
    core = dst // npc
    dloc = dst - core * npc
    win = dloc // WIN
    slot = dloc % WIN
    isA = (src < split).astype(np.int64)
    nk = num_cores * n_win * 2
    key = (core * n_win + win) * 2 + (1 - isA)
    order = np.argsort(key, kind="stable")
    sk = key[order]
    cnt = np.bincount(key, minlength=nk)
    cntA = cnt[0::2].reshape(num_cores, n_win)
    cntB = cnt[1::2].reshape(num_cores, n_win)
    t_a = max(1, int(-(-cntA.max() // 128)))
    t_b = max(1, int(-(-cntB.max() // 128)))
    tt = t_a + t_b
    ew = tt * 128
    e_core = n_win * ew
    mcols = tt * 5 + tt * 8

    grp_start = np.searchsorted(sk, np.arange(nk))
    pos = np.arange(E) - grp_start[sk]
    c_s = sk // (n_win * 2)
    w_s = (sk // 2) % n_win
    b_s = sk % 2
    dstpos = c_s * e_core + w_s * ew + b_s * (t_a * 128) + pos

    perm = np.full(num_cores * e_core, -1, np.int64)
    perm[dstpos] = order
    valid = perm >= 0
    pidx = np.where(valid, perm, 0)

    emb_p = (emb[pidx] * valid[:, None]).astype(BF16)     # [8EC, 10]
    ea_p = (eattr_f[pidx] * valid[:, None]).astype(BF16)  # [8EC, 4]
    blockpat = np.concatenate([np.zeros(t_a * 128, np.int64),
                               np.ones(t_b * 128, np.int64)])
    blockpat = np.tile(blockpat, num_cores * n_win)
    iv = np.where(valid, src[pidx] - split * blockpat, 0).astype(np.int16)
    sl_p = np.where(valid, slot[pidx], 0).astype(BF16)

    # device layouts
    embT_c = emb_p.reshape(num_cores, e_core, 10).transpose(0, 2, 1).copy()
    ea_c = ea_p.reshape(num_cores, n_win, tt, 128, 4).transpose(0, 3, 1, 2, 4) \
        .reshape(num_cores, 128, n_win, tt * 4).view(np.int16)
    dl_c = sl_p.reshape(num_cores, n_win, tt, 128).transpose(0, 3, 1, 2) \
        .reshape(num_cores, 128, n_win, tt).view(np.int16)
    ivr = iv.reshape(num_cores, n_win, tt * 128)
    ivA = ivr[:, :, :t_a * 128].reshape(num_cores, n_win, t_a * 8, 16)
    idxA_c = np.tile(ivA.transpose(0, 3, 1, 2).reshape(num_cores, 16, n_win, t_a * 8),
                     (1, 8, 1, 1))
    ivB = ivr[:, :, t_a * 128:].reshape(num_cores, n_win, t_b * 8, 16)
    idxB_c = np.tile(ivB.transpose(0, 3, 1, 2).reshape(num_cores, 16, n_win, t_b * 8),
                     (1, 8, 1, 1))

    meta_c = np.zeros((num_cores, 128, n_win, mcols), np.int16)
    meta_c[:, :, :, 0:tt * 4] = ea_c
    meta_c[:, :, :, tt * 4:tt * 5] = dl_c
    meta_c[:, :, :, tt * 5:tt * 5 + t_a * 8] = idxA_c
    meta_c[:, :, :, tt * 5 + t_a * 8:mcols] = idxB_c
    meta_c = meta_c.reshape(num_cores, 128, n_win * mcols)

    xwin_c = np.zeros((num_cores, 160, n_win * 128), BF16)
    for c in range(num_cores):
        xwin_c[c, :, :npc] = xaT[:, c * npc:(c + 1) * npc]

    in_maps = []
    for c in range(num_cores):
        in_maps.append({
            "xaT": xaT, "xwin": xwin_c[c],
            "embT": embT_c[c], "meta": meta_c[c],
            "Wbd": Wbd.astype(BF16), "W2b": W2b.astype(BF16),
            "Wfc1": Wfc1, "Wfc2b": Wfc2b,
            "Wbig": Wbig.astype(BF16), "iota": iota,
        })
    meta = dict(n_nodes=n_nodes, npc=npc, n_win=n_win, t_a=t_a, t_b=t_b,
                num_cores=num_cores, split=split)
    return in_maps, meta


def assemble(results, meta):
    """results: list of per-core dicts with 'out' [n_win*128,160] deint."""
    npc = meta["npc"]
    full = np.concatenate([r["out"][:npc] for r in results], axis=0)
    out = np.empty_like(full)
    out[:, :MUL0] = full[:, :MUL0]
    for d in range(3):
        out[:, MUL0 + d::3] = full[:, MUL0 + 32 * d:MUL0 + 32 * (d + 1)]
    return np.ascontiguousarray(out, dtype=np.float32)


_LAST_NC = None
_LAST_INMAPS = None
_LAST_META = None


def kernel(**inputs):
    global _LAST_RESULTS, _LAST_NC, _LAST_INMAPS, _LAST_META
    in_maps, meta = prepare(inputs)
    nc = build_program(meta["n_nodes"], meta["npc"], meta["n_win"],
                       meta["t_a"], meta["t_b"], meta["num_cores"],
                       split=meta["split"])
    _LAST_NC, _LAST_INMAPS, _LAST_META = nc, in_maps, meta
    res = bass_utils.run_bass_kernel_spmd(
        nc, in_maps, core_ids=list(range(meta["num_cores"])))
    _LAST_RESULTS = res
    return assemble(res.results, meta)


# revision 6
# speedup vs baseline: 1.5003x; 1.2422x over previous
"""Trainium2 Bass kernel for nn_Convolution (gnn_message_passing).

Strategy (no collectives needed):
  - Edges are sorted by destination node and partitioned across the 8 cores
    by dst range (each core owns N/8 destination nodes). Node features and
    weights are replicated; each core redundantly computes the lin1 table
    for all nodes (phase 1), then processes only edges destined to its own
    node slice (phase 2) and writes its slice of the output.
  - Phase 2 runs in "windows" of 128 destination slots. Per window:
    gather y=l[src] rows with dma_gather, radial MLP on PE, message build on
    DVE (bf16), segment-sum via selection-matrix matmuls accumulating in
    PSUM, then one fused (lin2 + self-interaction) matmul chain.
  - All matmuls are bf16 (fp32 LOW_HIGH matmuls are ~10x slower on PE).
  - All e3nn normalization constants and node_attr are folded into weights /
    edge attributes on the host.
"""

import sys

for _p in ("/opt/trn_rl_repo",):
    if _p not in sys.path:
        sys.path.insert(0, _p)

import numpy as np
import ml_dtypes

import concourse.bass as bass
import concourse.bacc as bacc
import concourse.mybir as mybir
import concourse.tile as tile
from concourse import bass_utils

BF16 = ml_dtypes.bfloat16

# Problem constants (hardcoded per contract)
N_NODES = 50000
N_EDGES = 800000
MUL0, MUL1 = 64, 32
N_BASIS, N_RADIAL = 10, 100
NUM_NEIGHBORS = 16.0
INV_SQRT3 = np.float32(1.0 / np.sqrt(3.0))
RELU_GAIN = np.float32(np.sqrt(2.0))
FAN_L2 = np.float32(np.sqrt(MUL0 + MUL1))

N_CORES = 8
SPLIT = 32768          # dma_gather idx is int16 -> split src tables
LROW = 256             # l-table row elems (bf16) -> 512B rows (256B-aligned)
WIN = 128              # dst slots per window

_LAST_RESULTS = None   # BassKernelResults of the most recent run (for test.py)


# --------------------------------------------------------------------------
# Device program
# --------------------------------------------------------------------------

def build_program(n_nodes, npc, n_win, t_a, t_b, num_cores, split=SPLIT):
    """Build the SPMD Bass program. npc = nodes per core."""
    tt = t_a + t_b
    ew = tt * 128            # padded edges per window
    e_core = n_win * ew
    mcols = tt * 5 + tt * 8  # meta: ea(tt*4) + dl(tt) + idxA(t_a*8) + idxB(t_b*8)
    oA = tt * 5
    oB = tt * 5 + t_a * 8
    f32, bf16, i16 = mybir.dt.float32, mybir.dt.bfloat16, mybir.dt.int16

    nc = bacc.Bacc("TRN2", target_bir_lowering=False, debug=False,
                   enable_asserts=False, num_devices=num_cores)

    # DRAM I/O (per-core data; weights replicated across cores)
    xaT = nc.dram_tensor("xaT", [160, n_nodes], bf16, kind="ExternalInput").ap()
    xwin = nc.dram_tensor("xwin", [160, n_win * 128], bf16, kind="ExternalInput").ap()
    embT = nc.dram_tensor("embT", [10, e_core], bf16, kind="ExternalInput").ap()
    meta = nc.dram_tensor("meta", [128, n_win * mcols], i16, kind="ExternalInput").ap()
    Wbd = nc.dram_tensor("Wbd", [128, 160], bf16, kind="ExternalInput").ap()
    W2b = nc.dram_tensor("W2b", [32, 160], bf16, kind="ExternalInput").ap()
    Wfc1 = nc.dram_tensor("Wfc1", [10, 100], bf16, kind="ExternalInput").ap()
    Wfc2b = nc.dram_tensor("Wfc2b", [100, 192], bf16, kind="ExternalInput").ap()
    Wbig = nc.dram_tensor("Wbig", [128, 5 * 160], bf16, kind="ExternalInput").ap()
    iota = nc.dram_tensor("iota", [128, 128], bf16, kind="ExternalInput").ap()
    out = nc.dram_tensor("out", [n_win * 128, 160], f32, kind="ExternalOutput").ap()

    mult = mybir.AluOpType.mult
    addop = mybir.AluOpType.add
    iseq = mybir.AluOpType.is_equal
    relu = mybir.ActivationFunctionType.Relu

    with tile.TileContext(nc) as tc:
        with (
            tc.tile_pool(name="const", bufs=1) as cpool,
            tc.tile_pool(name="ld", bufs=2) as ldpool,
            tc.tile_pool(name="ltab", bufs=1, space="DRAM") as dpool,
            tc.tile_pool(name="win", bufs=3) as wpool,
            tc.tile_pool(name="scr", bufs=2) as spool,
        ):
            # ---- constants to SBUF
            wbd_sb = cpool.tile([128, 160], bf16)
            nc.sync.dma_start(out=wbd_sb[:], in_=Wbd)
            w2b_sb = cpool.tile([32, 160], bf16)
            nc.sync.dma_start(out=w2b_sb[:], in_=W2b)
            wfc1_sb = cpool.tile([10, 100], bf16)
            nc.sync.dma_start(out=wfc1_sb[:], in_=Wfc1)
            wfc2_sb = cpool.tile([100, 192], bf16)
            nc.sync.dma_start(out=wfc2_sb[:], in_=Wfc2b)
            wbig_sb = cpool.tile([128, 5 * 160], bf16)
            nc.sync.dma_start(out=wbig_sb[:], in_=Wbig)
            iota_sb = cpool.tile([128, 128], bf16)
            nc.sync.dma_start(out=iota_sb[:], in_=iota)

            ltabA = dpool.tile([split, LROW], bf16)
            ltabB = dpool.tile([n_nodes - split, LROW], bf16)

            # ---- phase 1: l table (lin1 of all nodes), bf16 rows in DRAM
            # 2 node-tiles per PSUM bank; 2 fused block-diagonal matmuls per
            # tile; table split A/B so A-gathers can start before B is done.
            CH = 4096
            lps_ctx = tc.tile_pool(name="lps", bufs=3, space="PSUM")
            lpsum = lps_ctx.__enter__()
            for c0 in range(0, n_nodes, CH):
                cw = min(CH, n_nodes - c0)
                xa = ldpool.tile([128, CH], bf16, tag="xa")
                nc.sync.dma_start(out=xa[:, :cw], in_=xaT[0:128, c0:c0 + cw])
                xb = ldpool.tile([32, CH], bf16, tag="xb")
                nc.sync.dma_start(out=xb[:, :cw], in_=xaT[128:160, c0:c0 + cw])
                for t0 in range(0, cw, 256):
                    bw = min(256, cw - t0)
                    nb = (bw + 127) // 128
                    pl = lpsum.tile([128, 320], f32, tag="pl")
                    for b in range(nb):
                        nn_ = min(128, bw - b * 128)
                        nc.tensor.matmul(out=pl[:nn_, b * 160:b * 160 + 160],
                                         lhsT=xa[:, t0 + b * 128:t0 + b * 128 + nn_],
                                         rhs=wbd_sb[:],
                                         start=True, stop=False)
                        nc.tensor.matmul(out=pl[:nn_, b * 160:b * 160 + 160],
                                         lhsT=xb[:, t0 + b * 128:t0 + b * 128 + nn_],
                                         rhs=w2b_sb[:],
                                         start=False, stop=True)
                    lt = ldpool.tile([128, 320], bf16, tag="lt")
                    nc.scalar.copy(out=lt[:, :nb * 160], in_=pl[:, :nb * 160])
                    r0 = c0 + t0
                    tgt, ro = (ltabA, r0) if r0 < split else (ltabB, r0 - split)
                    nc.sync.dma_start(
                        out=tgt[ro:ro + bw, 0:160]
                            .rearrange("(b p) e -> p b e", p=128)
                        if bw == 256 else tgt[ro:ro + bw, 0:160],
                        in_=lt[:].rearrange("p (b e) -> p b e", e=160)[:, :nb, :]
                        if bw == 256 else lt[:bw, 0:160])
            lps_ctx.__exit__(None, None, None)

            # ---- phase 2: windows
            ps_ctx = tc.tile_pool(name="ps", bufs=2, space="PSUM")
            psum = ps_ctx.__enter__()
            n5 = (tt * 128 + 511) // 512
            for w in range(n_win):
                # loads (meta packs ea/dl/idxA/idxB in one i16 tensor)
                meta_w = wpool.tile([128, mcols], i16, tag="meta")
                nc.sync.dma_start(out=meta_w[:], in_=meta[:, w * mcols:(w + 1) * mcols])
                ea_w = meta_w[:, 0:tt * 4].bitcast(bf16)
                dl_w = meta_w[:, tt * 4:tt * 5].bitcast(bf16)
                y_w = wpool.tile([128, tt * 256], bf16, tag="y")
                nc.gpsimd.dma_gather(
                    y_w[:].rearrange("p (t e) -> p t e", e=256)[:, 0:t_a, :],
                    ltabA[:],
                    meta_w[:, oA:oA + t_a * 8],
                    t_a * 128, t_a * 128, 256, single_packet=False)
                nc.gpsimd.dma_gather(
                    y_w[:].rearrange("p (t e) -> p t e", e=256)[:, t_a:tt, :],
                    ltabB[:],
                    meta_w[:, oB:oB + t_b * 8],
                    t_b * 128, t_b * 128, 256, single_packet=False)
                emb_w = wpool.tile([10, tt * 128], bf16, tag="emb")
                nc.sync.dma_start(out=emb_w[:], in_=embT[:, w * ew:(w + 1) * ew])
                xw_a = wpool.tile([128, 128], bf16, tag="xwa")
                nc.sync.dma_start(out=xw_a[:], in_=xwin[0:128, w * 128:(w + 1) * 128])
                xw_b = wpool.tile([32, 128], bf16, tag="xwb")
                nc.sync.dma_start(out=xw_b[:], in_=xwin[128:160, w * 128:(w + 1) * 128])

                # radial MLP layer 1 (bf16), relu -> bf16
                hT = spool.tile([100, tt * 128], bf16, tag="hT")
                for c5 in range(n5):
                    ne = min(512, tt * 128 - c5 * 512)
                    ph = psum.tile([100, 512], f32, tag="ph", bufs=3)
                    nc.tensor.matmul(out=ph[:, :ne],
                                     lhsT=wfc1_sb[:],
                                     rhs=emb_w[:, c5 * 512:c5 * 512 + ne],
                                     start=True, stop=True)
                    nc.scalar.activation(hT[:, c5 * 512:c5 * 512 + ne], ph[:, :ne], relu)

                # radial layer 2 (bf16), two edge tiles per PSUM bank
                w_w = spool.tile([128, tt * 192], bf16, tag="ww")
                for t2_ in range(0, tt, 2):
                    ntl = min(2, tt - t2_)
                    pw = psum.tile([128, 384], f32, tag="pw")
                    for b in range(ntl):
                        nc.tensor.matmul(out=pw[:, b * 192:(b + 1) * 192],
                                         lhsT=hT[:, (t2_ + b) * 128:(t2_ + b + 1) * 128],
                                         rhs=wfc2_sb[:], start=True, stop=True)
                    nc.vector.tensor_copy(out=w_w[:, t2_ * 192:(t2_ + ntl) * 192],
                                          in_=pw[:, :ntl * 192])

                # selection matrices A (bf16 0/1), one batched iseq
                A_w = spool.tile([128, tt * 128], bf16, tag="A")
                nc.vector.tensor_tensor(
                    out=A_w[:].rearrange("p (t n) -> p t n", n=128),
                    in0=dl_w.rearrange("p (t o) -> p t o", o=1)
                        .to_broadcast([128, tt, 128]),
                    in1=iota_sb[:].rearrange("p (o n) -> p o n", o=1)
                        .to_broadcast([128, tt, 128]),
                    op=iseq)

                # messages M [128, tt, 384] bf16
                M_w = spool.tile([128, tt * 384], bf16, tag="M")
                y3 = y_w[:].rearrange("p (t e) -> p t e", e=256)
                w3 = w_w[:].rearrange("p (t e) -> p t e", e=192)
                m3 = M_w[:].rearrange("p (t e) -> p t e", e=384)
                ea3 = ea_w.rearrange("p (t e) -> p t e", e=4)

                def eb(col, n):
                    return ea3[:, :, col:col + 1].to_broadcast([128, tt, n])

                t0_s = spool.tile([128, tt * 64], bf16, tag="t0")
                t0v = t0_s[:].rearrange("p (t e) -> p t e", e=64)
                t1_s = spool.tile([128, tt * 64], bf16, tag="t1")
                t1v = t1_s[:].rearrange("p (t e) -> p t e", e=64)
                t2_s = spool.tile([128, tt * 32], bf16, tag="t2")
                t2v = t2_s[:].rearrange("p (t e) -> p t e", e=32)
                z_s = spool.tile([128, tt * 96], bf16, tag="z")
                zv = z_s[:].rearrange("p (t e) -> p t e", e=96)
                zz_s = spool.tile([128, tt * 32], bf16, tag="zz")
                zzv = zz_s[:].rearrange("p (t e) -> p t e", e=32)

                tt_ = nc.vector.tensor_tensor
                # m0 = (w0*y0)*e0
                tt_(out=t0v, in0=w3[:, :, 0:64], in1=y3[:, :, 0:64], op=mult)
                tt_(out=m3[:, :, 0:64], in0=t0v, in1=eb(0, 64), op=mult)
                # m1_d = (w1*y0)*e1d
                tt_(out=t1v, in0=w3[:, :, 64:128], in1=y3[:, :, 0:64], op=mult)
                for d in range(3):
                    tt_(out=m3[:, :, 64 + 64 * d:128 + 64 * d],
                        in0=t1v, in1=eb(1 + d, 64), op=mult)
                # m2_d = (w2*e0)*y1_d
                tt_(out=t2v, in0=w3[:, :, 128:160], in1=eb(0, 32), op=mult)
                for d in range(3):
                    tt_(out=m3[:, :, 256 + 32 * d:288 + 32 * d],
                        in0=t2v, in1=y3[:, :, 64 + 32 * d:96 + 32 * d], op=mult)
                # m3 = w3 * sum_d(y1_d*e1_d)
                for d in range(3):
                    tt_(out=zv[:, :, 32 * d:32 * (d + 1)],
                        in0=y3[:, :, 64 + 32 * d:96 + 32 * d], in1=eb(1 + d, 32), op=mult)
                tt_(out=zzv, in0=zv[:, :, 0:32], in1=zv[:, :, 32:64], op=addop)
                tt_(out=zzv, in0=zzv, in1=zv[:, :, 64:96], op=addop)
                tt_(out=m3[:, :, 352:384], in0=zzv, in1=w3[:, :, 160:192], op=mult)

                # segment-sum: sT[f, slot] += M_chunk.T @ A  (3 chunks, acc over t)
                pst = psum.tile([128, 384], f32, tag="pst")
                for ch in range(3):
                    for t in range(tt):
                        nc.tensor.matmul(
                            out=pst[:, ch * 128:(ch + 1) * 128],
                            lhsT=m3[:, t, ch * 128:(ch + 1) * 128],
                            rhs=A_w[:, t * 128:(t + 1) * 128],
                            start=(t == 0), stop=(t == tt - 1))
                sT_sb = spool.tile([128, 384], bf16, tag="sT")
                nc.vector.tensor_copy(out=sT_sb[:], in_=pst[:])

                # fused lin2 + self-interaction: out[slot, 0:160], all bf16
                po = psum.tile([128, 160], f32, tag="po", bufs=1)
                for ch in range(3):
                    nc.tensor.matmul(out=po[:],
                                     lhsT=sT_sb[:, ch * 128:(ch + 1) * 128],
                                     rhs=wbig_sb[:, ch * 160:(ch + 1) * 160],
                                     start=(ch == 0), stop=False)
                nc.tensor.matmul(out=po[:], lhsT=xw_a[:],
                                 rhs=wbig_sb[:, 480:640],
                                 start=False, stop=False)
                nc.tensor.matmul(out=po[:], lhsT=xw_b[:],
                                 rhs=wbig_sb[0:32, 640:800],
                                 start=False, stop=True)
                o_sb = spool.tile([128, 160], f32, tag="o")
                nc.vector.tensor_copy(out=o_sb[:], in_=po[:])
                nc.sync.dma_start(out=out[w * 128:(w + 1) * 128, :], in_=o_sb[:])
            ps_ctx.__exit__(None, None, None)

    nc.compile()
    return nc


# --------------------------------------------------------------------------
# Host-side preparation
# --------------------------------------------------------------------------

def prepare(inputs, n_nodes=N_NODES, num_cores=N_CORES, split=SPLIT):
    npc = n_nodes // num_cores
    n_win = (npc + WIN - 1) // WIN

    f32 = np.float32
    node_input = np.asarray(inputs["node_input"], f32)
    node_attr = np.asarray(inputs["node_attr"], f32)
    edge_attr = np.asarray(inputs["edge_attr"], f32)
    emb = np.asarray(inputs["edge_length_embedded"], f32)
    src = np.asarray(inputs["edge_src"], np.int64)
    dst = np.asarray(inputs["edge_dst"], np.int64)
    E = src.shape[0]

    # fold node_attr into node features; de-interleave x1 by d
    xa = node_input * node_attr
    xg = np.concatenate([xa[:, :MUL0], xa[:, MUL0 + 0::3],
                         xa[:, MUL0 + 1::3], xa[:, MUL0 + 2::3]], axis=1)
    xaT = np.ascontiguousarray(xg.T).astype(BF16)         # [160, n_nodes]

    # fold node_attr[dst] into edge_attr
    eattr_f = edge_attr * node_attr[dst, 0][:, None]

    # weights with norm constants folded; phase-1 block-diagonal layout
    Wl10 = np.asarray(inputs["W_l1_0"], f32) / np.sqrt(MUL0).astype(f32)
    Wl11 = np.asarray(inputs["W_l1_1"], f32) / np.sqrt(MUL1).astype(f32)
    Wbd = np.zeros((128, 160), f32)
    Wbd[0:64, 0:64] = Wl10
    Wbd[64:96, 64:96] = Wl11
    Wbd[96:128, 96:128] = Wl11
    W2b = np.zeros((32, 160), f32)
    W2b[:, 128:160] = Wl11
    Wfc1 = (np.asarray(inputs["W_fc1"], f32) / np.sqrt(np.float32(N_BASIS))).astype(BF16)
    Wfc2b = (np.asarray(inputs["W_fc2"], f32) * (RELU_GAIN / np.sqrt(np.float32(N_RADIAL)))).astype(BF16)

    c2 = np.float32(0.5 / np.sqrt(NUM_NEIGHBORS) / FAN_L2)
    W2cat = np.zeros((384, 160), f32)
    W2cat[0:64, 0:64] = np.asarray(inputs["W_l2_00"], f32) * c2
    W2cat[352:384, 0:64] = np.asarray(inputs["W_l2_10"], f32) * c2 * INV_SQRT3
    for d in range(3):
        W2cat[64 + 64 * d:128 + 64 * d, 64 + 32 * d:96 + 32 * d] = \
            np.asarray(inputs["W_l2_01"], f32) * c2
        W2cat[256 + 32 * d:288 + 32 * d, 64 + 32 * d:96 + 32 * d] = \
            np.asarray(inputs["W_l2_11"], f32) * c2
    Wsi = np.zeros((160, 160), f32)
    Wsi[0:64, 0:64] = np.asarray(inputs["W_si0"], f32) / np.sqrt(MUL0).astype(f32)
    for d in range(3):
        Wsi[64 + 32 * d:96 + 32 * d, 64 + 32 * d:96 + 32 * d] = \
            np.asarray(inputs["W_si1"], f32) / np.sqrt(MUL1).astype(f32)
    Wfull = np.vstack([W2cat, Wsi])                       # [544, 160]
    Wbig = np.zeros((128, 5 * 160), f32)
    for ch in range(4):
        Wbig[:, ch * 160:(ch + 1) * 160] = Wfull[ch * 128:(ch + 1) * 128]
    Wbig[0:32, 640:800] = Wfull[512:544]

    iota = np.broadcast_to(np.arange(128, dtype=f32), (128, 128)).astype(BF16)

    # ---- edge partition: (core, window, srcblock), stable sorted
    core = dst // npc
    dloc = dst - core * npc
    win = dloc // WIN
    slot = dloc % WIN
    isA = (src < split).astype(np.int64)
    nk = num_cores * n_win * 2
    key = (core * n_win + win) * 2 + (1 - isA)
    order = np.argsort(key, kind="stable")
    sk = key[order]
    cnt = np.bincount(key, minlength=nk)
    cntA = cnt[0::2].reshape(num_cores, n_win)
    cntB = cnt[1::2].reshape(num_cores, n_win)
    t_a = max(1, int(-(-cntA.max() // 128)))
    t_b = max(1, int(-(-cntB.max() // 128)))
    tt = t_a + t_b
    ew = tt * 128
    e_core = n_win * ew
    mcols = tt * 5 + tt * 8

    grp_start = np.searchsorted(sk, np.arange(nk))
    pos = np.arange(E) - grp_start[sk]
    c_s = sk // (n_win * 2)
    w_s = (sk // 2) % n_win
    b_s = sk % 2
    dstpos = c_s * e_core + w_s * ew + b_s * (t_a * 128) + pos

    perm = np.full(num_cores * e_core, -1, np.int64)
    perm[dstpos] = order
    valid = perm >= 0
    pidx = np.where(valid, perm, 0)

    emb_p = (emb[pidx] * valid[:, None]).astype(BF16)     # [8EC, 10]
    ea_p = (eattr_f[pidx] * valid[:, None]).astype(BF16)  # [8EC, 4]
    blockpat = np.concatenate([np.zeros(t_a * 128, np.int64),
                               np.ones(t_b * 128, np.int64)])
    blockpat = np.tile(blockpat, num_cores * n_win)
    iv = np.where(valid, src[pidx] - split * blockpat, 0).astype(np.int16)
    sl_p = np.where(valid, slot[pidx], 0).astype(BF16)

    # device layouts
    embT_c = emb_p.reshape(num_cores, e_core, 10).transpose(0, 2, 1).copy()
    ea_c = ea_p.reshape(num_cores, n_win, tt, 128, 4).transpose(0, 3, 1, 2, 4) \
        .reshape(num_cores, 128, n_win, tt * 4).view(np.int16)
    dl_c = sl_p.reshape(num_cores, n_win, tt, 128).transpose(0, 3, 1, 2) \
        .reshape(num_cores, 128, n_win, tt).view(np.int16)
    ivr = iv.reshape(num_cores, n_win, tt * 128)
    ivA = ivr[:, :, :t_a * 128].reshape(num_cores, n_win, t_a * 8, 16)
    idxA_c = np.tile(ivA.transpose(0, 3, 1, 2).reshape(num_cores, 16, n_win, t_a * 8),
                     (1, 8, 1, 1))
    ivB = ivr[:, :, t_a * 128:].reshape(num_cores, n_win, t_b * 8, 16)
    idxB_c = np.tile(ivB.transpose(0, 3, 1, 2).reshape(num_cores, 16, n_win, t_b * 8),
                     (1, 8, 1, 1))

    meta_c = np.zeros((num_cores, 128, n_win, mcols), np.int16)
    meta_c[:, :, :, 0:tt * 4] = ea_c
    meta_c[:, :, :, tt * 4:tt * 5] = dl_c
    meta_c[:, :, :, tt * 5:tt * 5 + t_a * 8] = idxA_c
    meta_c[:, :, :, tt * 5 + t_a * 8:mcols] = idxB_c
    meta_c = meta_c.reshape(num_cores, 128, n_win * mcols)

    xwin_c = np.zeros((num_cores, 160, n_win * 128), BF16)
    for c in range(num_cores):
        xwin_c[c, :, :npc] = xaT[:, c * npc:(c + 1) * npc]

    in_maps = []
    for c in range(num_cores):
        in_maps.append({
            "xaT": xaT, "xwin": xwin_c[c],
            "embT": embT_c[c], "meta": meta_c[c],
            "Wbd": Wbd.astype(BF16), "W2b": W2b.astype(BF16),
            "Wfc1": Wfc1, "Wfc2b": Wfc2b,
            "Wbig": Wbig.astype(BF16), "iota": iota,
        })
    meta = dict(n_nodes=n_nodes, npc=npc, n_win=n_win, t_a=t_a, t_b=t_b,
                num_cores=num_cores, split=split)
    return in_maps, meta


def assemble(results, meta):
    """results: list of per-core dicts with 'out' [n_win*128,160] deint."""
    npc = meta["npc"]
    full = np.concatenate([r["out"][:npc] for r in results], axis=0)
    out = np.empty_like(full)
    out[:, :MUL0] = full[:, :MUL0]
    for d in range(3):
        out[:, MUL0 + d::3] = full[:, MUL0 + 32 * d:MUL0 + 32 * (d + 1)]
    return np.ascontiguousarray(out, dtype=np.float32)


_LAST_NC = None
_LAST_INMAPS = None
_LAST_META = None


def kernel(**inputs):
    global _LAST_RESULTS, _LAST_NC, _LAST_INMAPS, _LAST_META
    in_maps, meta = prepare(inputs)
    nc = build_program(meta["n_nodes"], meta["npc"], meta["n_win"],
                       meta["t_a"], meta["t_b"], meta["num_cores"],
                       split=meta["split"])
    _LAST_NC, _LAST_INMAPS, _LAST_META = nc, in_maps, meta
    res = bass_utils.run_bass_kernel_spmd(
        nc, in_maps, core_ids=list(range(meta["num_cores"])))
    _LAST_RESULTS = res
    return assemble(res.results, meta)
